# revision 13
# baseline (speedup 1.0000x reference)
"""DLRM-style DistTrainModel forward + BCE loss on 8 Trainium2 NeuronCores.

Strategy: pure data parallelism over the 4096 batch (512 samples/core).
The full embedding table (26 x 100000 x 128 f32) is replicated to every
core's DRAM; each core gathers 26*512 rows locally (no collectives).

Per-core pipeline (all feature-major [feature, batch] layouts):
  bottom MLP (13->512->256->128, relu)               : PE matmuls, K/M tiles
  embedding gather                                    : 4 indirect DMAs,
        [128 samples, 26 idx/partition] -> [128, 26*128] batch-major
  transpose to feature-major                          : PE transposes (128x128)
  pairwise interactions Z_s = T_s^T T_s (27x27/sample): 512 small PE matmuls
        reading strided APs (stride 128) from the transposed feature tile
  Z relayout                                          : per-PSUM-bank strided
        copies into a [108, 7*512] k-major tile (groups of 7 j-columns
        stacked along partitions)
  top MLP (479->1024->1024->512->256->1)              : PE matmuls; layer-0
        weight matrix host-side re-expanded to consume the full 27x27 Z
        (symmetrized halves) instead of the lower triangle
  final sigmoid + BCE                                 : host (float64), from
        the returned [1,512] pre-bias logits per core
"""

import sys
import types

import numpy as np


def _install_axon_hooks_shim():
    """This image's antenv package lacks axon_hooks; concourse imports it
    unguarded when trace=True under axon. Provide it, wired to the
    trn_agent_boot ctypes NTFF profiler when available."""
    try:
        import antenv.axon_hooks  # noqa: F401
        return
    except ImportError:
        pass
    mod = types.ModuleType("antenv.axon_hooks")
    _state = {"hook": None}
    mod.set_axon_ntff_profile_hook = lambda h: _state.__setitem__("hook", h)
    mod.get_axon_ntff_profile_hook = lambda: _state["hook"]
    sys.modules["antenv.axon_hooks"] = mod
    try:
        import antenv
        antenv.axon_hooks = mod
    except ImportError:
        pass
    try:
        from trn_agent_boot.trn_boot import _ntff_profile_via_ctypes
        hook = _ntff_profile_via_ctypes("/opt/axon/libaxon_pjrt.so")
        if hook is not None:
            mod.set_axon_ntff_profile_hook(hook)
    except Exception:
        pass


_install_axon_hooks_shim()

import concourse.bass as bass
import concourse.bacc as bacc
import concourse.mybir as mybir
import concourse.tile as tile
from concourse.masks import make_identity
from concourse.bass_utils import run_bass_kernel_spmd

NCORES = 8
B = 4096
BL = B // NCORES          # 512 samples per core
NT = 26                   # embedding tables
D = 128                   # embedding dim
V = 100000                # vocab per table
NF = NT + 1               # 27 interaction features (dense x + 26 tables)
BLK = 128                 # samples per gather/transpose block
NBLK = BL // BLK          # 4
SPB = 16                  # samples per interaction PSUM bank
NBANK = BL // SPB         # 32
F32 = mybir.dt.float32
I32 = mybir.dt.int32

# top layer-0: Z consumed as 7 column-blocks of j-columns, 4 groups of
# js stacked on partitions: group g holds j in [7g, 7g+7) -> rows 32g+i
# (32-aligned start partitions; gap rows 32g+27..32g+31 are zeroed)
ZJ_GROUPS = 4
ZJ_PER_G = 7
GOFF = 32
ZROWS = 3 * GOFF + NF     # 123 partition rows of the Z k-major tile


def _zk(jp):
    """contraction size of z column-block jp (last block misses j=27)"""
    return 2 * GOFF + NF if jp == ZJ_PER_G - 1 else ZROWS


def build_nc(v=V):
    nc = bacc.Bacc("TRN2", target_bir_lowering=False)

    emb = nc.dram_tensor("emb", [NT * v, D], F32, kind="ExternalInput")
    idx = nc.dram_tensor("idx", [BLK, NBLK * NT], I32, kind="ExternalInput")
    xT = nc.dram_tensor("xT", [13, BL], F32, kind="ExternalInput")
    bw0T = nc.dram_tensor("bw0T", [13, 512], F32, kind="ExternalInput")
    bw1T = nc.dram_tensor("bw1T", [128, 4 * 256], F32, kind="ExternalInput")
    bw2T = nc.dram_tensor("bw2T", [128, 2 * 128], F32, kind="ExternalInput")
    w0x = nc.dram_tensor("w0x", [128, 1024], F32, kind="ExternalInput")
    w0z = nc.dram_tensor("w0z", [ZROWS, ZJ_PER_G * 1024], F32, kind="ExternalInput")
    w1 = nc.dram_tensor("w1", [128, 8 * 1024], F32, kind="ExternalInput")
    w2 = nc.dram_tensor("w2", [128, 8 * 512], F32, kind="ExternalInput")
    w3 = nc.dram_tensor("w3", [128, 4 * 256], F32, kind="ExternalInput")
    w4 = nc.dram_tensor("w4", [128, 2], F32, kind="ExternalInput")
    bb0 = nc.dram_tensor("bb0", [128, 4], F32, kind="ExternalInput")
    bb1 = nc.dram_tensor("bb1", [128, 2], F32, kind="ExternalInput")
    bb2 = nc.dram_tensor("bb2", [128, 1], F32, kind="ExternalInput")
    tb0 = nc.dram_tensor("tb0", [128, 8], F32, kind="ExternalInput")
    tb1 = nc.dram_tensor("tb1", [128, 8], F32, kind="ExternalInput")
    tb2 = nc.dram_tensor("tb2", [128, 4], F32, kind="ExternalInput")
    tb3 = nc.dram_tensor("tb3", [128, 2], F32, kind="ExternalInput")
    z4 = nc.dram_tensor("z4", [1, BL], F32, kind="ExternalOutput")

    with tile.TileContext(nc) as tc:
        with (
            tc.tile_pool(name="wpool", bufs=1) as wp,
            tc.tile_pool(name="acts", bufs=1) as ap,
            tc.tile_pool(name="big", bufs=2) as bp,
            tc.tile_pool(name="zpool", bufs=1) as zp,
            tc.tile_pool(name="mlp_ps", bufs=2, space="PSUM") as mps,
            tc.tile_pool(name="tr_ps", bufs=3, space="PSUM") as tps,
            tc.tile_pool(name="z_ps", bufs=3, space="PSUM") as zps,
        ):
            # ---- resident weights / constants ----
            def _load(dram, shape, dtype=F32):
                t = wp.tile(shape, dtype, tag=dram.name)
                nc.sync.dma_start(out=t[: shape[0], :], in_=dram[:])
                return t

            ident = wp.tile([128, 128], F32, tag="ident")
            make_identity(nc, ident[:])
            idx_sb = _load(idx, [BLK, NBLK * NT], I32)
            xT_sb = _load(xT, [13, BL])
            bw0T_sb = _load(bw0T, [13, 512])
            bw1T_sb = _load(bw1T, [128, 1024])
            bw2T_sb = _load(bw2T, [128, 256])
            w0x_sb = _load(w0x, [128, 1024])
            w0z_sb = _load(w0z, [ZROWS, ZJ_PER_G * 1024])
            w1_sb = _load(w1, [128, 8192])
            w2_sb = _load(w2, [128, 4096])
            w3_sb = _load(w3, [128, 1024])
            w4_sb = _load(w4, [128, 2])
            bb0_sb = _load(bb0, [128, 4])
            bb1_sb = _load(bb1, [128, 2])
            bb2_sb = _load(bb2, [128, 1])
            tb0_sb = _load(tb0, [128, 8])
            tb1_sb = _load(tb1, [128, 8])
            tb2_sb = _load(tb2, [128, 4])
            tb3_sb = _load(tb3, [128, 2])

            relu = mybir.ActivationFunctionType.Relu

            # ---- bottom MLP (feature-major, N=512) ----
            a0 = ap.tile([128, 4 * BL], F32, tag="a0")     # 13 -> 512
            for m in range(4):
                ps = mps.tile([128, BL], F32, space="PSUM")
                nc.tensor.matmul(out=ps[:], lhsT=bw0T_sb[:13, m * 128:(m + 1) * 128],
                                 rhs=xT_sb[:13, :], start=True, stop=True)
                nc.scalar.activation(out=a0[:, m * BL:(m + 1) * BL], in_=ps[:],
                                     func=relu, bias=bb0_sb[:, m:m + 1])
            a1 = ap.tile([128, 2 * BL], F32, tag="a1")     # 512 -> 256
            for m in range(2):
                ps = mps.tile([128, BL], F32, space="PSUM")
                for k in range(4):
                    nc.tensor.matmul(
                        out=ps[:],
                        lhsT=bw1T_sb[:, k * 256 + m * 128:k * 256 + (m + 1) * 128],
                        rhs=a0[:, k * BL:(k + 1) * BL],
                        start=(k == 0), stop=(k == 3))
                nc.scalar.activation(out=a1[:, m * BL:(m + 1) * BL], in_=ps[:],
                                     func=relu, bias=bb1_sb[:, m:m + 1])
            a2 = ap.tile([128, BL], F32, tag="a2")         # 256 -> 128
            ps = mps.tile([128, BL], F32, space="PSUM")
            for k in range(2):
                nc.tensor.matmul(out=ps[:],
                                 lhsT=bw2T_sb[:, k * 128:(k + 1) * 128],
                                 rhs=a1[:, k * BL:(k + 1) * BL],
                                 start=(k == 0), stop=(k == 1))
            nc.scalar.activation(out=a2[:], in_=ps[:], func=relu,
                                 bias=bb2_sb[:, 0:1])

            # ---- Z k-major tile: [32g+i, jp*512 + s] = Z_s[i, 7g+jp] ----
            zc2 = zp.tile([ZROWS, ZJ_PER_G * BL], F32, tag="zc2")
            nc.vector.memset(zc2[:], 0.0)

            # ---- per-block: gather + transpose + interactions ----
            for blk in range(NBLK):
                ebig = bp.tile([BLK, NT * D], F32, tag="ebig")
                nc.gpsimd.indirect_dma_start(
                    out=ebig[:], out_offset=None, in_=emb[:],
                    in_offset=bass.IndirectOffsetOnAxis(
                        ap=idx_sb[:, blk * NT:(blk + 1) * NT], axis=0))

                tt = bp.tile([128, NF * BLK], F32, tag="tt")
                # feature 0 = dense bottom-MLP output (already feature-major)
                nc.vector.tensor_copy(
                    out=tt[:, 0:BLK], in_=a2[:, blk * BLK:(blk + 1) * BLK])
                for t in range(NT):
                    tr = tps.tile([128, 128], F32, space="PSUM")
                    nc.tensor.transpose(out=tr[:], in_=ebig[:, t * D:(t + 1) * D],
                                        identity=ident[:])
                    nc.vector.tensor_copy(
                        out=tt[:, (1 + t) * BLK:(2 + t) * BLK], in_=tr[:])

                # interactions: Z_s = tt[:, s::128].T @ tt[:, s::128]
                tt_str = tt[:].rearrange("p (t s) -> p s t", s=BLK)
                for w in range(BLK // SPB):                 # 8 banks per block
                    zt = zps.tile([NF, SPB * NF], F32, space="PSUM")
                    for sl in range(SPB):
                        s = w * SPB + sl
                        apx = tt_str[:, s, :]               # [128, 27] stride 128
                        nc.tensor.matmul(out=zt[:, sl * NF:(sl + 1) * NF],
                                         lhsT=apx, rhs=apx, start=True, stop=True)
                    # relayout: bank -> zc2 (one copy per j-group)
                    gw = blk * (BLK // SPB) + w             # global bank id
                    zt_r = zt[:].rearrange("p (s j) -> p j s", j=NF)
                    zc2_r = zc2[:].rearrange("p (j s) -> p j s", s=BL)
                    for g in range(ZJ_GROUPS):
                        cnt = ZJ_PER_G if g < 3 else ZJ_PER_G - 1
                        nc.any.tensor_copy(
                            out=zc2_r[GOFF * g:GOFF * g + NF, 0:cnt,
                                      gw * SPB:(gw + 1) * SPB],
                            in_=zt_r[:, ZJ_PER_G * g:ZJ_PER_G * g + cnt, :])

            # ---- top MLP ----
            t0 = bp.tile([128, 8 * BL], F32, tag="ebig")    # 479(857) -> 1024
            for m in range(8):
                ps = mps.tile([128, BL], F32, space="PSUM")
                nc.tensor.matmul(out=ps[:], lhsT=w0x_sb[:, m * 128:(m + 1) * 128],
                                 rhs=a2[:], start=True, stop=False)
                for jp in range(ZJ_PER_G):
                    K = _zk(jp)
                    nc.tensor.matmul(
                        out=ps[:],
                        lhsT=w0z_sb[0:K, jp * 1024 + m * 128:jp * 1024 + (m + 1) * 128],
                        rhs=zc2[0:K, jp * BL:(jp + 1) * BL],
                        start=False, stop=(jp == ZJ_PER_G - 1))
                nc.scalar.activation(out=t0[:, m * BL:(m + 1) * BL], in_=ps[:],
                                     func=relu, bias=tb0_sb[:, m:m + 1])

            t1 = bp.tile([128, 8 * BL], F32, tag="tt")      # 1024 -> 1024
            for m in range(8):
                ps = mps.tile([128, BL], F32, space="PSUM")
                for k in range(8):
                    nc.tensor.matmul(
                        out=ps[:],
                        lhsT=w1_sb[:, k * 1024 + m * 128:k * 1024 + (m + 1) * 128],
                        rhs=t0[:, k * BL:(k + 1) * BL],
                        start=(k == 0), stop=(k == 7))
                nc.scalar.activation(out=t1[:, m * BL:(m + 1) * BL], in_=ps[:],
                                     func=relu, bias=tb1_sb[:, m:m + 1])

            t2 = ap.tile([128, 4 * BL], F32, tag="a0")      # 1024 -> 512
            for m in range(4):
                ps = mps.tile([128, BL], F32, space="PSUM")
                for k in range(8):
                    nc.tensor.matmul(
                        out=ps[:],
                        lhsT=w2_sb[:, k * 512 + m * 128:k * 512 + (m + 1) * 128],
                        rhs=t1[:, k * BL:(k + 1) * BL],
                        start=(k == 0), stop=(k == 7))
                nc.scalar.activation(out=t2[:, m * BL:(m + 1) * BL], in_=ps[:],
                                     func=relu, bias=tb2_sb[:, m:m + 1])

            t3 = ap.tile([128, 2 * BL], F32, tag="a1")      # 512 -> 256
            for m in range(2):
                ps = mps.tile([128, BL], F32, space="PSUM")
                for k in range(4):
                    nc.tensor.matmul(
                        out=ps[:],
                        lhsT=w3_sb[:, k * 256 + m * 128:k * 256 + (m + 1) * 128],
                        rhs=t2[:, k * BL:(k + 1) * BL],
                        start=(k == 0), stop=(k == 3))
                nc.scalar.activation(out=t3[:, m * BL:(m + 1) * BL], in_=ps[:],
                                     func=relu, bias=tb3_sb[:, m:m + 1])

            # 256 -> 1 (no bias, no sigmoid: host side)
            ps = mps.tile([1, BL], F32, space="PSUM")
            for k in range(2):
                nc.tensor.matmul(out=ps[:], lhsT=w4_sb[:, k:k + 1],
                                 rhs=t3[:, k * BL:(k + 1) * BL],
                                 start=(k == 0), stop=(k == 1))
            z4_sb = ap.tile([1, BL], F32, tag="z4")
            nc.vector.tensor_copy(out=z4_sb[:], in_=ps[:])
            nc.sync.dma_start(out=z4[:], in_=z4_sb[:])

    nc.finalize()
    return nc


def _ktile(wT, K, M):
    """[K, M] -> [128, (K/128)*M] with k-tiles side by side"""
    kt = K // 128
    return np.ascontiguousarray(
        wT.reshape(kt, 128, M).transpose(1, 0, 2).reshape(128, kt * M))


def _bias_tiles(b, mt):
    return np.ascontiguousarray(b.reshape(mt, 128).T)


def prep_shared(bw0, bb0, bw1, bb1, bw2, bb2,
                tw0, tb0, tw1, tb1, tw2, tb2, tw3, tb3):
    f = np.float32
    shared = {
        "bw0T": np.ascontiguousarray(bw0.astype(f).T),                # [13, 512]
        "bw1T": _ktile(bw1.astype(f).T, 512, 256),
        "bw2T": _ktile(bw2.astype(f).T, 256, 128),
        "w0x": np.ascontiguousarray(tw0[:, :128].astype(f).T),        # [128,1024]
        "w1": _ktile(tw1.astype(f).T, 1024, 1024),
        "w2": _ktile(tw2.astype(f).T, 1024, 512),
        "w3": _ktile(tw3.astype(f).T, 512, 256),
        "bb0": _bias_tiles(bb0.astype(f), 4),
        "bb1": _bias_tiles(bb1.astype(f), 2),
        "bb2": _bias_tiles(bb2.astype(f), 1),
        "tb0": _bias_tiles(tb0.astype(f), 8),
        "tb1": _bias_tiles(tb1.astype(f), 8),
        "tb2": _bias_tiles(tb2.astype(f), 4),
        "tb3": _bias_tiles(tb3.astype(f), 2),
    }
    # expanded layer-0 interaction weights: symmetrized over the full 27x27
    li, lj = np.tril_indices(NF, -1)
    wsym = np.zeros((1024, NF, NF), f)
    wz = tw0[:, 128:479].astype(f)                                    # [1024, 351]
    wsym[:, li, lj] = 0.5 * wz
    wsym[:, lj, li] = 0.5 * wz
    w0z_sb = np.zeros((ZROWS, ZJ_PER_G, 1024), f)
    for g in range(ZJ_GROUPS):
        for jp in range(ZJ_PER_G):
            j = ZJ_PER_G * g + jp
            if j >= NF:
                continue
            w0z_sb[GOFF * g:GOFF * g + NF, jp, :] = wsym[:, :, j].T
    shared["w0z"] = np.ascontiguousarray(w0z_sb.reshape(ZROWS, ZJ_PER_G * 1024))
    return shared


def prep_w4(tw4):
    w4T = tw4.astype(np.float32).T                                    # [256, 1]
    return np.ascontiguousarray(w4T.reshape(2, 128, 1).transpose(1, 0, 2).reshape(128, 2))


def prep_core_inputs(dense_x, gidx, core):
    sl = slice(core * BL, (core + 1) * BL)
    idx_c = np.empty((BLK, NBLK * NT), np.int32)
    for blk in range(NBLK):
        lo = core * BL + blk * BLK
        idx_c[:, blk * NT:(blk + 1) * NT] = gidx[:, lo:lo + BLK].T
    return {
        "idx": idx_c,
        "xT": np.ascontiguousarray(dense_x[sl].astype(np.float32).T),
    }


def host_loss(z4_all, tb4, target):
    z = z4_all.astype(np.float64) + float(np.asarray(tb4).reshape(-1)[0])
    p = 1.0 / (1.0 + np.exp(-z))
    logp = np.maximum(np.log(p), -100.0)
    log1mp = np.maximum(np.log1p(-p), -100.0)
    t = np.asarray(target).reshape(-1).astype(np.float64)
    loss = -np.mean(t * logp + (1.0 - t) * log1mp)
    return np.float32(loss)


_NC_CACHE = None
LAST_RESULTS = None


def kernel(dense_x, lS_i, target, emb,
           bw0, bb0, bw1, bb1, bw2, bb2,
           tw0, tb0, tw1, tb1, tw2, tb2, tw3, tb3, tw4, tb4):
    global _NC_CACHE
    if _NC_CACHE is None:
        _NC_CACHE = build_nc()
    nc = _NC_CACHE

    emb_flat = np.ascontiguousarray(
        np.asarray(emb, dtype=np.float32).reshape(NT * V, D))
    gidx = (np.asarray(lS_i, dtype=np.int64)
            + (np.arange(NT, dtype=np.int64) * V)[:, None]).astype(np.int32)

    shared = prep_shared(bw0, bb0, bw1, bb1, bw2, bb2,
                         tw0, tb0, tw1, tb1, tw2, tb2, tw3, tb3)
    shared["w4"] = prep_w4(tw4)
    shared["emb"] = emb_flat

    in_maps = []
    for c in range(NCORES):
        m = dict(shared)
        m.update(prep_core_inputs(dense_x, gidx, c))
        in_maps.append(m)

    res = run_bass_kernel_spmd(nc, in_maps, core_ids=list(range(NCORES)))
    global LAST_RESULTS
    LAST_RESULTS = res
    z4_all = np.concatenate([r["z4"].reshape(-1) for r in res.results])
    return host_loss(z4_all, tb4, target)


# revision 15
# speedup vs baseline: 2.7999x; 2.7999x over previous
"""DLRM-style DistTrainModel forward + BCE loss on 8 Trainium2 NeuronCores.

Strategy: pure data parallelism over the 4096 batch (512 samples/core).
The full embedding table (26 x 100000 x 128, cast bf16) is replicated to
every core's DRAM; each core gathers 26*512 rows locally (no collectives).

Per-core pipeline (all feature-major [feature, batch] layouts, bf16
operands with fp32 PSUM accumulation):
  embedding gather          : 4 indirect DMAs, [128 samples, 26 idx] ->
                              [128, 26*128] batch-major bf16
  bottom MLP 13->512->256->128 (relu)
  transpose to feature-major: PE transposes (128x128)
  interactions Z_s = T_s^T T_s (27x27 per sample): 512 small PE matmuls
                              on strided APs (stride 128)
  Z relayout                : per-PSUM-bank strided copies into a
                              [123, 7*512] k-major tile (4 groups of 7
                              j-columns at 32-aligned partition offsets)
  top MLP 479->1024->1024->512->256->1 : layer-0 weights host-expanded to
                              consume the full 27x27 Z (symmetrized)
  final bias+sigmoid+BCE    : host (float64) from returned [1,512] logits
"""

import sys
import types

import numpy as np
import ml_dtypes


def _install_axon_hooks_shim():
    """This image's antenv package lacks axon_hooks; concourse imports it
    unguarded when trace=True under axon. Provide it, wired to the
    trn_agent_boot ctypes NTFF profiler when available."""
    try:
        import antenv.axon_hooks  # noqa: F401
        return
    except ImportError:
        pass
    mod = types.ModuleType("antenv.axon_hooks")
    _state = {"hook": None}
    mod.set_axon_ntff_profile_hook = lambda h: _state.__setitem__("hook", h)
    mod.get_axon_ntff_profile_hook = lambda: _state["hook"]
    sys.modules["antenv.axon_hooks"] = mod
    try:
        import antenv
        antenv.axon_hooks = mod
    except ImportError:
        pass
    try:
        from trn_agent_boot.trn_boot import _ntff_profile_via_ctypes
        hook = _ntff_profile_via_ctypes("/opt/axon/libaxon_pjrt.so")
        if hook is not None:
            mod.set_axon_ntff_profile_hook(hook)
    except Exception:
        pass


_install_axon_hooks_shim()

import concourse.bass as bass
import concourse.bacc as bacc
import concourse.mybir as mybir
import concourse.tile as tile
from concourse.masks import make_identity
from concourse.bass_utils import run_bass_kernel_spmd

NCORES = 8
B = 4096
BL = B // NCORES          # 512 samples per core
NT = 26
D = 128
V = 100000
NF = NT + 1               # 27 interaction features
BLK = 128                 # samples per gather/transpose block
NBLK = BL // BLK          # 4
SPB = 16                  # samples per interaction PSUM bank
F32 = mybir.dt.float32
BF16 = mybir.dt.bfloat16
I32 = mybir.dt.int32
NPBF = ml_dtypes.bfloat16

ZJ_GROUPS = 4
ZJ_PER_G = 7
GOFF = 32
ZROWS = 3 * GOFF + NF     # 123


def _zk(jp):
    return 2 * GOFF + NF if jp == ZJ_PER_G - 1 else ZROWS


def build_nc(v=V):
    nc = bacc.Bacc("TRN2", target_bir_lowering=False)

    emb = nc.dram_tensor("emb", [NT * v, D], BF16, kind="ExternalInput")
    idx = nc.dram_tensor("idx", [BLK, NBLK * NT], I32, kind="ExternalInput")
    xT = nc.dram_tensor("xT", [13, BL], BF16, kind="ExternalInput")
    bw0T = nc.dram_tensor("bw0T", [13, 512], BF16, kind="ExternalInput")
    bw1T = nc.dram_tensor("bw1T", [128, 4 * 256], BF16, kind="ExternalInput")
    bw2T = nc.dram_tensor("bw2T", [128, 2 * 128], BF16, kind="ExternalInput")
    w0x = nc.dram_tensor("w0x", [128, 1024], BF16, kind="ExternalInput")
    w0z = nc.dram_tensor("w0z", [ZROWS, ZJ_PER_G * 1024], BF16, kind="ExternalInput")
    w1 = nc.dram_tensor("w1", [128, 8 * 1024], BF16, kind="ExternalInput")
    w2 = nc.dram_tensor("w2", [128, 8 * 512], BF16, kind="ExternalInput")
    w3 = nc.dram_tensor("w3", [128, 4 * 256], BF16, kind="ExternalInput")
    w4 = nc.dram_tensor("w4", [128, 2], BF16, kind="ExternalInput")
    bb0 = nc.dram_tensor("bb0", [128, 4], F32, kind="ExternalInput")
    bb1 = nc.dram_tensor("bb1", [128, 2], F32, kind="ExternalInput")
    bb2 = nc.dram_tensor("bb2", [128, 1], F32, kind="ExternalInput")
    tb0 = nc.dram_tensor("tb0", [128, 8], F32, kind="ExternalInput")
    tb1 = nc.dram_tensor("tb1", [128, 8], F32, kind="ExternalInput")
    tb2 = nc.dram_tensor("tb2", [128, 4], F32, kind="ExternalInput")
    tb3 = nc.dram_tensor("tb3", [128, 2], F32, kind="ExternalInput")
    z4 = nc.dram_tensor("z4", [1, BL], F32, kind="ExternalOutput")

    with tile.TileContext(nc) as tc:
        with (
            tc.tile_pool(name="wpool", bufs=1) as wp,
            tc.tile_pool(name="acts", bufs=1) as ap,
            tc.tile_pool(name="big", bufs=2) as bp,
            tc.tile_pool(name="zpool", bufs=1) as zp,
            tc.tile_pool(name="mlp_ps", bufs=2, space="PSUM") as mps,
            tc.tile_pool(name="tr_ps", bufs=3, space="PSUM") as tps,
            tc.tile_pool(name="z_ps", bufs=3, space="PSUM") as zps,
        ):
            def _load(dram, shape, dtype=BF16):
                t = wp.tile(shape, dtype, tag=dram.name)
                nc.sync.dma_start(out=t[: shape[0], :], in_=dram[:])
                return t

            # ---- indices + gathers first: embedding rows stream in while
            # weights load and the bottom MLP runs ----
            idx_sb = _load(idx, [BLK, NBLK * NT], I32)
            ebigs = []
            for blk in range(NBLK):
                ebig = bp.tile([BLK, NT * D], BF16, tag="ebig")
                nc.gpsimd.indirect_dma_start(
                    out=ebig[:], out_offset=None, in_=emb[:],
                    in_offset=bass.IndirectOffsetOnAxis(
                        ap=idx_sb[:, blk * NT:(blk + 1) * NT], axis=0))
                ebigs.append(ebig)

            ident = wp.tile([128, 128], BF16, tag="ident")
            make_identity(nc, ident[:])
            xT_sb = _load(xT, [13, BL])
            bw0T_sb = _load(bw0T, [13, 512])
            bw1T_sb = _load(bw1T, [128, 1024])
            bw2T_sb = _load(bw2T, [128, 256])
            bb0_sb = _load(bb0, [128, 4], F32)
            bb1_sb = _load(bb1, [128, 2], F32)
            bb2_sb = _load(bb2, [128, 1], F32)

            relu = mybir.ActivationFunctionType.Relu

            # ---- bottom MLP (feature-major, N=512) ----
            a0 = ap.tile([128, 4 * BL], BF16, tag="a0")
            for m in range(4):
                ps = mps.tile([128, BL], F32, space="PSUM")
                nc.tensor.matmul(out=ps[:], lhsT=bw0T_sb[:13, m * 128:(m + 1) * 128],
                                 rhs=xT_sb[:13, :], start=True, stop=True)
                nc.scalar.activation(out=a0[:, m * BL:(m + 1) * BL], in_=ps[:],
                                     func=relu, bias=bb0_sb[:, m:m + 1])
            a1 = ap.tile([128, 2 * BL], BF16, tag="a1")
            for m in range(2):
                ps = mps.tile([128, BL], F32, space="PSUM")
                for k in range(4):
                    nc.tensor.matmul(
                        out=ps[:],
                        lhsT=bw1T_sb[:, k * 256 + m * 128:k * 256 + (m + 1) * 128],
                        rhs=a0[:, k * BL:(k + 1) * BL],
                        start=(k == 0), stop=(k == 3))
                nc.scalar.activation(out=a1[:, m * BL:(m + 1) * BL], in_=ps[:],
                                     func=relu, bias=bb1_sb[:, m:m + 1])
            a2 = ap.tile([128, BL], BF16, tag="a2")
            ps = mps.tile([128, BL], F32, space="PSUM")
            for k in range(2):
                nc.tensor.matmul(out=ps[:],
                                 lhsT=bw2T_sb[:, k * 128:(k + 1) * 128],
                                 rhs=a1[:, k * BL:(k + 1) * BL],
                                 start=(k == 0), stop=(k == 1))
            nc.scalar.activation(out=a2[:], in_=ps[:], func=relu,
                                 bias=bb2_sb[:, 0:1])

            # top weights stream in during the interaction phase
            w0x_sb = _load(w0x, [128, 1024])
            w0z_sb = _load(w0z, [ZROWS, ZJ_PER_G * 1024])
            w1_sb = _load(w1, [128, 8192])
            w2_sb = _load(w2, [128, 4096])
            w3_sb = _load(w3, [128, 1024])
            w4_sb = _load(w4, [128, 2])
            tb0_sb = _load(tb0, [128, 8], F32)
            tb1_sb = _load(tb1, [128, 8], F32)
            tb2_sb = _load(tb2, [128, 4], F32)
            tb3_sb = _load(tb3, [128, 2], F32)

            # ---- Z k-major tile: [32g+i, jp*512 + s] = Z_s[i, 7g+jp] ----
            zc2 = zp.tile([ZROWS, ZJ_PER_G * BL], BF16, tag="zc2")
            nc.vector.memset(zc2[:], 0.0)

            # ---- per-block: transpose + interactions ----
            for blk in range(NBLK):
                ebig = ebigs[blk]
                tt = bp.tile([128, NF * BLK], BF16, tag="tt")
                nc.vector.tensor_copy(
                    out=tt[:, 0:BLK], in_=a2[:, blk * BLK:(blk + 1) * BLK])
                for t in range(NT):
                    tr = tps.tile([128, 128], BF16, space="PSUM")
                    nc.tensor.transpose(out=tr[:], in_=ebig[:, t * D:(t + 1) * D],
                                        identity=ident[:])
                    nc.vector.tensor_copy(
                        out=tt[:, (1 + t) * BLK:(2 + t) * BLK], in_=tr[:])

                tt_str = tt[:].rearrange("p (t s) -> p s t", s=BLK)
                for w in range(BLK // SPB):
                    zt = zps.tile([NF, SPB * NF], F32, space="PSUM")
                    for sl in range(SPB):
                        s = w * SPB + sl
                        apx = tt_str[:, s, :]
                        nc.tensor.matmul(out=zt[:, sl * NF:(sl + 1) * NF],
                                         lhsT=apx, rhs=apx, start=True, stop=True)
                    gw = blk * (BLK // SPB) + w
                    zt_r = zt[:].rearrange("p (s j) -> p j s", j=NF)
                    zc2_r = zc2[:].rearrange("p (j s) -> p j s", s=BL)
                    for g in range(ZJ_GROUPS):
                        cnt = ZJ_PER_G if g < 3 else ZJ_PER_G - 1
                        nc.any.tensor_copy(
                            out=zc2_r[GOFF * g:GOFF * g + NF, 0:cnt,
                                      gw * SPB:(gw + 1) * SPB],
                            in_=zt_r[:, ZJ_PER_G * g:ZJ_PER_G * g + cnt, :])

            # ---- top MLP ----
            t0 = bp.tile([128, 8 * BL], BF16, tag="ebig")
            for m in range(8):
                ps = mps.tile([128, BL], F32, space="PSUM")
                nc.tensor.matmul(out=ps[:], lhsT=w0x_sb[:, m * 128:(m + 1) * 128],
                                 rhs=a2[:], start=True, stop=False)
                for jp in range(ZJ_PER_G):
                    K = _zk(jp)
                    nc.tensor.matmul(
                        out=ps[:],
                        lhsT=w0z_sb[0:K, jp * 1024 + m * 128:jp * 1024 + (m + 1) * 128],
                        rhs=zc2[0:K, jp * BL:(jp + 1) * BL],
                        start=False, stop=(jp == ZJ_PER_G - 1))
                nc.scalar.activation(out=t0[:, m * BL:(m + 1) * BL], in_=ps[:],
                                     func=relu, bias=tb0_sb[:, m:m + 1])

            t1 = bp.tile([128, 8 * BL], BF16, tag="tt")
            for m in range(8):
                ps = mps.tile([128, BL], F32, space="PSUM")
                for k in range(8):
                    nc.tensor.matmul(
                        out=ps[:],
                        lhsT=w1_sb[:, k * 1024 + m * 128:k * 1024 + (m + 1) * 128],
                        rhs=t0[:, k * BL:(k + 1) * BL],
                        start=(k == 0), stop=(k == 7))
                nc.scalar.activation(out=t1[:, m * BL:(m + 1) * BL], in_=ps[:],
                                     func=relu, bias=tb1_sb[:, m:m + 1])

            t2 = ap.tile([128, 4 * BL], BF16, tag="a0")
            for m in range(4):
                ps = mps.tile([128, BL], F32, space="PSUM")
                for k in range(8):
                    nc.tensor.matmul(
                        out=ps[:],
                        lhsT=w2_sb[:, k * 512 + m * 128:k * 512 + (m + 1) * 128],
                        rhs=t1[:, k * BL:(k + 1) * BL],
                        start=(k == 0), stop=(k == 7))
                nc.scalar.activation(out=t2[:, m * BL:(m + 1) * BL], in_=ps[:],
                                     func=relu, bias=tb2_sb[:, m:m + 1])

            t3 = ap.tile([128, 2 * BL], BF16, tag="a1")
            for m in range(2):
                ps = mps.tile([128, BL], F32, space="PSUM")
                for k in range(4):
                    nc.tensor.matmul(
                        out=ps[:],
                        lhsT=w3_sb[:, k * 256 + m * 128:k * 256 + (m + 1) * 128],
                        rhs=t2[:, k * BL:(k + 1) * BL],
                        start=(k == 0), stop=(k == 3))
                nc.scalar.activation(out=t3[:, m * BL:(m + 1) * BL], in_=ps[:],
                                     func=relu, bias=tb3_sb[:, m:m + 1])

            ps = mps.tile([1, BL], F32, space="PSUM")
            for k in range(2):
                nc.tensor.matmul(out=ps[:], lhsT=w4_sb[:, k:k + 1],
                                 rhs=t3[:, k * BL:(k + 1) * BL],
                                 start=(k == 0), stop=(k == 1))
            z4_sb = ap.tile([1, BL], F32, tag="z4")
            nc.vector.tensor_copy(out=z4_sb[:], in_=ps[:])
            nc.sync.dma_start(out=z4[:], in_=z4_sb[:])

    nc.finalize()
    return nc


def _ktile(wT, K, M):
    kt = K // 128
    return np.ascontiguousarray(
        wT.reshape(kt, 128, M).transpose(1, 0, 2).reshape(128, kt * M))


def _bias_tiles(bb, mt):
    return np.ascontiguousarray(bb.astype(np.float32).reshape(mt, 128).T)


def prep_shared(bw0, bb0, bw1, bb1, bw2, bb2,
                tw0, tb0, tw1, tb1, tw2, tb2, tw3, tb3, tw4):
    bf = NPBF
    shared = {
        "bw0T": np.ascontiguousarray(bw0.T.astype(bf)),
        "bw1T": _ktile(bw1.T, 512, 256).astype(bf),
        "bw2T": _ktile(bw2.T, 256, 128).astype(bf),
        "w0x": np.ascontiguousarray(tw0[:, :128].T.astype(bf)),
        "w1": _ktile(tw1.T, 1024, 1024).astype(bf),
        "w2": _ktile(tw2.T, 1024, 512).astype(bf),
        "w3": _ktile(tw3.T, 512, 256).astype(bf),
        "w4": np.ascontiguousarray(
            tw4.T.reshape(2, 128, 1).transpose(1, 0, 2).reshape(128, 2).astype(bf)),
        "bb0": _bias_tiles(bb0, 4),
        "bb1": _bias_tiles(bb1, 2),
        "bb2": _bias_tiles(bb2, 1),
        "tb0": _bias_tiles(tb0, 8),
        "tb1": _bias_tiles(tb1, 8),
        "tb2": _bias_tiles(tb2, 4),
        "tb3": _bias_tiles(tb3, 2),
    }
    li, lj = np.tril_indices(NF, -1)
    wsym = np.zeros((1024, NF, NF), np.float32)
    wz = tw0[:, 128:479].astype(np.float32)
    wsym[:, li, lj] = 0.5 * wz
    wsym[:, lj, li] = 0.5 * wz
    w0z_sb = np.zeros((ZROWS, ZJ_PER_G, 1024), np.float32)
    for g in range(ZJ_GROUPS):
        for jp in range(ZJ_PER_G):
            j = ZJ_PER_G * g + jp
            if j >= NF:
                continue
            w0z_sb[GOFF * g:GOFF * g + NF, jp, :] = wsym[:, :, j].T
    shared["w0z"] = np.ascontiguousarray(
        w0z_sb.reshape(ZROWS, ZJ_PER_G * 1024).astype(bf))
    return shared


def prep_core_inputs(dense_x, gidx, core):
    sl = slice(core * BL, (core + 1) * BL)
    idx_c = np.empty((BLK, NBLK * NT), np.int32)
    for blk in range(NBLK):
        lo = core * BL + blk * BLK
        idx_c[:, blk * NT:(blk + 1) * NT] = gidx[:, lo:lo + BLK].T
    return {
        "idx": idx_c,
        "xT": np.ascontiguousarray(dense_x[sl].T.astype(NPBF)),
    }


def host_loss(z4_all, tb4, target):
    z = z4_all.astype(np.float64) + float(np.asarray(tb4).reshape(-1)[0])
    p = 1.0 / (1.0 + np.exp(-z))
    logp = np.maximum(np.log(p), -100.0)
    log1mp = np.maximum(np.log1p(-p), -100.0)
    t = np.asarray(target).reshape(-1).astype(np.float64)
    loss = -np.mean(t * logp + (1.0 - t) * log1mp)
    return np.float32(loss)


_NC_CACHE = None
LAST_RESULTS = None


def kernel(dense_x, lS_i, target, emb,
           bw0, bb0, bw1, bb1, bw2, bb2,
           tw0, tb0, tw1, tb1, tw2, tb2, tw3, tb3, tw4, tb4):
    global _NC_CACHE, LAST_RESULTS
    if _NC_CACHE is None:
        _NC_CACHE = build_nc()
    nc = _NC_CACHE

    emb_flat = np.ascontiguousarray(
        np.asarray(emb).reshape(NT * V, D).astype(NPBF))
    gidx = (np.asarray(lS_i, dtype=np.int64)
            + (np.arange(NT, dtype=np.int64) * V)[:, None]).astype(np.int32)

    shared = prep_shared(bw0, bb0, bw1, bb1, bw2, bb2,
                         tw0, tb0, tw1, tb1, tw2, tb2, tw3, tb3, tw4)
    shared["emb"] = emb_flat

    in_maps = []
    for c in range(NCORES):
        m = dict(shared)
        m.update(prep_core_inputs(dense_x, gidx, c))
        in_maps.append(m)

    res = run_bass_kernel_spmd(nc, in_maps, core_ids=list(range(NCORES)))
    LAST_RESULTS = res
    z4_all = np.concatenate([r["z4"].reshape(-1) for r in res.results])
    return host_loss(z4_all, tb4, target)
